# revision 41
# baseline (speedup 1.0000x reference)
"""Trainium2 Bass kernel for nn_DeltaLag (LSTM encoder + lagged cross-attention
top-k + MLP head), distributed over 8 NeuronCores.

Key structure (v2):
- Stocks are split 375/core; each core's local stock order is PERMUTED so the
  distinct-target stocks come first (<=256 per core, asserted).  That makes
  the query exchange transpose-free and the merge block-contiguous.
- target_idx has repeats: only ~1904 of 3000 targets are distinct.  Scores /
  top-k / merge run over 2048 padded distinct targets (16 tiles); the final
  [3000] output is a host-side gather.
- LSTM: first T0=20 steps use bf16 matmuls (errors decay ~0.5/step through
  the forget gates; validated zero top-5 flips at bf16-level noise), last 20
  steps full fp32, A/B stock-half interleaved to hide the per-step serial
  chain.  Scores/keys/queries stay fp32: the #5-#6 score gap is as small as
  7e-8, so any score noise above ~1e-7 flips selections.
- Candidate exchange is an AllToAll (each core only receives its own 256
  targets' candidates), q exchange is an AllGather of [128,128] fp32 blocks;
  both are split even/odd so half of each overlaps the score computation.
"""

import sys

sys.path.insert(0, "/opt/trn_rl_repo")

import numpy as np

import concourse.bacc as bacc
import concourse.mybir as mybir
import concourse.tile as tile
from concourse.bass import IndirectOffsetOnAxis
from concourse.bass_utils import run_bass_kernel_spmd
from concourse.masks import make_identity

F32 = mybir.dt.float32
BF16 = mybir.dt.bfloat16
U32 = mybir.dt.uint32
AF = mybir.ActivationFunctionType
ALU = mybir.AluOpType

S, F, N, L, LMAX, K = 3000, 6, 128, 40, 10, 5
ND = 8                      # cores
SS = S // ND                # stocks per core
COLS = SS * LMAX            # score columns per core
B = 256                     # distinct-target slots per core
NPOS = ND * B               # padded distinct targets (2048)
NT = NPOS // 128            # target tiles (16)
T0 = 20                     # LSTM steps run with bf16 matmuls
CH = 1024                   # score PSUM chunk (2 banks)


def build_program():
    nc = bacc.Bacc("TRN2", target_bir_lowering=False, debug=False,
                   enable_asserts=True, num_devices=ND)

    # ---- I/O ----
    # xt is split: steps < T0 arrive pre-cast to bf16 (cheap matmuls), the
    # rest in fp32.  The bf16 side carries TWO ones-rows so the bias can be
    # folded as bias_hi + bias_lo (bf16 pair, ~1e-6 accurate; the residual
    # decays through the forget gates).  The fp32 side folds bias exactly.
    d_xtb = nc.dram_tensor("xtb", [F + 2, SS * T0], BF16, kind="ExternalInput")
    d_xt = nc.dram_tensor("xt", [F + 1, SS * (L - T0)], F32, kind="ExternalInput")
    d_wihb = nc.dram_tensor("wihb_t", [F + 2, 4 * N], BF16, kind="ExternalInput")
    d_whhb = nc.dram_tensor("whhb_t", [N, 4 * N], BF16, kind="ExternalInput")
    d_wih = nc.dram_tensor("wih_t", [F + 1, 4 * N], F32, kind="ExternalInput")
    d_whh = nc.dram_tensor("whh_t", [N, 4 * N], F32, kind="ExternalInput")
    d_wqt = nc.dram_tensor("wq_t", [N, N], F32, kind="ExternalInput")
    d_wkt = nc.dram_tensor("wk_t", [N, N], F32, kind="ExternalInput")
    d_nuq = nc.dram_tensor("negu_q", [1, N], F32, kind="ExternalInput")
    d_nuk = nc.dram_tensor("negu_k", [1, N], F32, kind="ExternalInput")
    d_invt = nc.dram_tensor("invt", [1, 1], F32, kind="ExternalInput")
    d_slo = nc.dram_tensor("selflo", [128, NT], F32, kind="ExternalInput")
    d_xzb = nc.dram_tensor("xzb", [S * LMAX, 8], F32, kind="ExternalInput")
    d_w1t = nc.dram_tensor("w1_t", [2 * F, 64], F32, kind="ExternalInput")
    d_w2t = nc.dram_tensor("w2_t", [64, 32], F32, kind="ExternalInput")
    d_w3t = nc.dram_tensor("w3_t", [32, 1], F32, kind="ExternalInput")
    d_b1 = nc.dram_tensor("b1c", [64, 1], F32, kind="ExternalInput")
    d_b2 = nc.dram_tensor("b2c", [32, 1], F32, kind="ExternalInput")
    d_b3 = nc.dram_tensor("b3c", [1, 1], F32, kind="ExternalInput")

    d_y = nc.dram_tensor("y", [B, 1], F32, kind="ExternalOutput")

    # q exchange and candidate exchange are split even/odd so the second half
    # of each overlaps compute: even-tile scores start after the first (half
    # size) AllGather; even-tile candidates exchange + merge run underneath
    # the odd-tile scores.
    d_qta = nc.dram_tensor("qbl_ta", [N, 128], F32)
    d_qtb = nc.dram_tensor("qbl_tb", [N, 128], F32)
    d_qaa = nc.dram_tensor("qba_ta", [ND * N, 128], F32, addr_space="Shared")
    d_qab = nc.dram_tensor("qba_tb", [ND * N, 128], F32, addr_space="Shared")
    d_cle = nc.dram_tensor("cand_le", [ND * 128, 16], U32)
    d_clo = nc.dram_tensor("cand_lo", [ND * 128, 16], U32)
    d_cae = nc.dram_tensor("cand_ae", [ND * 128, 16], U32)
    d_cao = nc.dram_tensor("cand_ao", [ND * 128, 16], U32)

    groups = [list(range(ND))]

    with tile.TileContext(nc) as tc:
        cpool = tc.alloc_tile_pool(name="const", bufs=1)
        big = tc.alloc_tile_pool(name="big", bufs=1)

        # ---- constants / params to SBUF ----
        ident = cpool.tile([128, 128], F32)
        make_identity(nc, ident[:])
        ones1 = cpool.tile([1, 128], F32)
        nc.vector.memset(ones1[:], 1.0)
        onesf = cpool.tile([128, 128], F32)
        nc.vector.memset(onesf[:], 1.0)

        def load(pool, dram, shape, dtype=F32):
            t = pool.tile(shape, dtype, tag=f"ld_{dram.name}")
            nc.sync.dma_start(out=t[:], in_=dram[:, :])
            return t

        xtb = big.tile([F + 2, SS * T0], BF16)
        xt = big.tile([F + 1, SS * (L - T0)], F32)
        wih = load(cpool, d_wih, [F + 1, 4 * N])
        whh = load(cpool, d_whh, [N, 4 * N])
        wihb = load(cpool, d_wihb, [F + 2, 4 * N], BF16)
        whhb = load(cpool, d_whhb, [N, 4 * N], BF16)
        wqt = load(cpool, d_wqt, [N, N])
        wkt = load(cpool, d_wkt, [N, N])
        nuq = load(cpool, d_nuq, [1, N])
        nuk = load(cpool, d_nuk, [1, N])
        invt = load(cpool, d_invt, [1, 1])
        slo = load(cpool, d_slo, [128, NT])
        w1t = load(cpool, d_w1t, [2 * F, 64])
        w2t = load(cpool, d_w2t, [64, 32])
        w3t = load(cpool, d_w3t, [32, 1])
        b1c = load(cpool, d_b1, [64, 1])
        b2c = load(cpool, d_b2, [32, 1])
        b3c = load(cpool, d_b3, [1, 1])
        for c in range(4):
            nc.sync.dma_start(out=xtb[:, c * 5 * SS:(c + 1) * 5 * SS],
                              in_=d_xtb[:, c * 5 * SS:(c + 1) * 5 * SS])
        for c in range(4):
            nc.sync.dma_start(out=xt[:, c * 5 * SS:(c + 1) * 5 * SS],
                              in_=d_xt[:, c * 5 * SS:(c + 1) * 5 * SS])

        invtb = cpool.tile([128, 1], F32)
        nc.gpsimd.partition_broadcast(invtb[:], invt[:], channels=128)

        iota_u = cpool.tile([128, 64], U32)
        nc.gpsimd.iota(iota_u[:], pattern=[[1, 64]], base=0, channel_multiplier=0)
        iota_f = cpool.tile([128, 64], F32)
        nc.vector.tensor_copy(iota_f[:], iota_u[:])
        base_u = cpool.tile([128, 64], U32)
        nc.gpsimd.iota(base_u[:], pattern=[[COLS, 8], [0, 8]], base=0,
                       channel_multiplier=0)
        base_f = cpool.tile([128, 64], F32)
        nc.vector.tensor_copy(base_f[:], base_u[:])

        # rank-1 LN-fold correction matrices: rows n, cols p -> -u[p]/128
        with tc.tile_pool(name="ppre", bufs=1, space="PSUM") as ppre:
            uqo = cpool.tile([128, 128], F32)
            uko = cpool.tile([128, 128], F32)
            pq = ppre.tile([128, 128], F32, space="PSUM")
            nc.tensor.matmul(out=pq[:], lhsT=ones1[:], rhs=nuq[:], start=True, stop=True)
            nc.scalar.activation(uqo[:], pq[:], AF.Copy)
            pk = ppre.tile([128, 128], F32, space="PSUM")
            nc.tensor.matmul(out=pk[:], lhsT=ones1[:], rhs=nuk[:], start=True, stop=True)
            nc.scalar.activation(uko[:], pk[:], AF.Copy)

        # ---- Phase 1: LSTM over the 375 local stocks ----
        # h,c layout [n=128, s]; last-10 hidden states land in hsave[n, s*10+k].
        # Gate columns in wih/whh are host-permuted to [i, f, o, g]; the bias
        # is folded into the xproj matmul via xt's constant-1 row.
        # Steps < T0 run the matmuls in fp32r (error decays through the forget
        # gates); the last steps are full fp32.
        # Two independent stock-halves (A: cols 0:188, B: 188:375) pipeline so
        # the per-step serial chain of one half hides under the other's
        # engine work.  All 4 gate pre-activations of a half live in ONE
        # 2-bank PSUM tile (i,f,o at 188-col slices, g at 564+), so i/f/o
        # share a single packed Sigmoid.
        GW = 188
        halves = [(0, GW), (GW, SS - GW)]
        hsave = big.tile([128, COLS], F32)
        with tc.tile_pool(name="lstm_sb", bufs=2) as lsb, \
             tc.tile_pool(name="lstm_c", bufs=2) as lcp, \
             tc.tile_pool(name="lstm_ps", bufs=2, space="PSUM") as lps:
            h_prev = []
            c_prev = []
            for hh, (c0, w) in enumerate(halves):
                h0 = lsb.tile([128, GW], BF16, tag=f"h0_{hh}")
                nc.vector.memset(h0[:, :w], 0.0)
                c0t = lcp.tile([128, GW], F32, tag=f"c_{hh}")
                nc.vector.memset(c0t[:, :w], 0.0)
                h_prev.append(h0[:, :w])
                c_prev.append(c0t)
            for t in range(L):
                early = t < T0
                wih_t = wihb if early else wih
                whh_t = whhb if early else whh
                for hh, (c0, w) in enumerate(halves):
                    xt_t = (xtb[:, t * SS + c0:t * SS + c0 + w] if early
                            else xt[:, (t - T0) * SS + c0:(t - T0) * SS + c0 + w])
                    P1 = lps.tile([128, 512], F32, tag=f"gif{hh}", space="PSUM")
                    P2 = lps.tile([128, 512], F32, tag=f"gog{hh}", space="PSUM")
                    slots = [(P1, 0), (P1, w), (P2, 0), (P2, w)]  # i, f, o, g
                    for g in range(4):
                        Pg, o0 = slots[g]
                        nc.tensor.matmul(out=Pg[:, o0:o0 + w],
                                         lhsT=wih_t[:, g * N:(g + 1) * N],
                                         rhs=xt_t, start=True, stop=False)
                        nc.tensor.matmul(out=Pg[:, o0:o0 + w],
                                         lhsT=whh_t[:, g * N:(g + 1) * N],
                                         rhs=h_prev[hh], start=False, stop=True)
                    sifo = lsb.tile([128, 3 * GW], F32, tag=f"sifo{hh}")
                    nc.scalar.activation(sifo[:, :2 * w], P1[:, 0:2 * w], AF.Sigmoid)
                    si = sifo[:, 0:w]
                    sf = sifo[:, w:2 * w]
                    t1 = lsb.tile([128, GW], F32, tag=f"t1{hh}")
                    if early:
                        # g-gate weights host-scaled x2: one packed sigmoid
                        # yields [so, sg] with tanh(g) = 2*sg - 1, so
                        # t1 = si*tanh(g) = 2*(si*sg) - si (on Pool, off the
                        # DVE chain)
                        sog = lsb.tile([128, 2 * GW], F32, tag=f"sog{hh}")
                        nc.scalar.activation(sog[:, :2 * w], P2[:, 0:2 * w],
                                             AF.Sigmoid)
                        so = sog[:, 0:w]
                        u = lsb.tile([128, GW], F32, tag=f"u{hh}")
                        nc.gpsimd.tensor_tensor(out=u[:, :w], in0=si,
                                                in1=sog[:, w:2 * w], op=ALU.mult)
                        # TensorScalarPtr is not a legal Pool opcode on HW;
                        # keep this one on DVE
                        nc.vector.scalar_tensor_tensor(
                            out=t1[:, :w], in0=u[:, :w], scalar=2.0, in1=si,
                            op0=ALU.mult, op1=ALU.subtract)
                    else:
                        nc.scalar.activation(sifo[:, 2 * w:3 * w], P2[:, 0:w],
                                             AF.Sigmoid)
                        tg = lsb.tile([128, GW], F32, tag=f"tg{hh}")
                        nc.scalar.activation(tg[:, :w], P2[:, w:2 * w], AF.Tanh)
                        so = sifo[:, 2 * w:3 * w]
                        nc.vector.tensor_tensor(out=t1[:, :w], in0=si,
                                                in1=tg[:, :w], op=ALU.mult)
                    c2 = lsb.tile([128, GW], F32, tag=f"c2{hh}")
                    nc.gpsimd.tensor_tensor(out=c2[:, :w], in0=sf,
                                            in1=c_prev[hh][:, :w], op=ALU.mult)
                    c_new = lcp.tile([128, GW], F32, tag=f"c_{hh}")
                    nc.vector.tensor_tensor(out=c_new[:, :w], in0=c2[:, :w],
                                            in1=t1[:, :w], op=ALU.add)
                    th = lsb.tile([128, GW], F32, tag=f"th{hh}")
                    nc.scalar.activation(th[:, :w], c_new[:, :w], AF.Tanh)
                    if t >= L - LMAX:
                        k = t - (L - LMAX)
                        h_out = hsave[:, k + LMAX * c0:
                                      k + LMAX * (c0 + w - 1) + 1:LMAX]
                        nc.gpsimd.tensor_tensor(out=h_out, in0=so, in1=th[:, :w],
                                                op=ALU.mult)
                        h_prev[hh] = h_out
                    else:
                        hdt = BF16 if (t + 1 < T0) else F32
                        hn = lsb.tile([128, GW], hdt,
                                      tag=f"hn{hh}" if hdt == F32 else f"hnb{hh}")
                        nc.gpsimd.tensor_tensor(out=hn[:, :w], in0=so,
                                                in1=th[:, :w], op=ALU.mult)
                        h_prev[hh] = hn[:, :w]
                    c_prev[hh] = c_new

        # ---- Phase 2: queries for the local distinct-target slots ----
        # Local stock order is host-permuted so slots 0..B-1 are the distinct
        # targets owned by this core; q comes out feature-major, so the
        # exchange needs no transposes at all.
        with tc.tile_pool(name="q_sb", bufs=2) as qsb, \
             tc.tile_pool(name="q_ps", bufs=2, space="PSUM") as qps:
            h39 = hsave[:, (LMAX - 1):(B * LMAX):LMAX]
            pyq = qps.tile([128, B], F32, tag="yq", space="PSUM")
            nc.tensor.matmul(out=pyq[:], lhsT=wqt[:], rhs=h39, start=True, stop=False)
            nc.tensor.matmul(out=pyq[:], lhsT=uqo[:], rhs=h39, start=False, stop=True)
            yq = qsb.tile([128, B], F32, tag="yq_sb")
            nc.scalar.activation(yq[:], pyq[:], AF.Copy)
            y2q = qsb.tile([128, B], F32, tag="y2q")
            nc.scalar.activation(y2q[:], pyq[:], AF.Square)
            psq = qps.tile([128, B], F32, tag="sq", space="PSUM")
            nc.tensor.matmul(out=psq[:], lhsT=onesf[:], rhs=y2q[:], start=True, stop=True)
            sq = qsb.tile([128, B], F32, tag="sqq")
            nc.scalar.sqrt(sq[:], psq[:])
            ri = qsb.tile([128, B], F32, tag="riq")
            nc.vector.reciprocal(ri[:], sq[:])
            qn1 = qsb.tile([128, B], F32, tag="qn1")
            nc.vector.tensor_tensor(out=qn1[:], in0=yq[:], in1=ri[:], op=ALU.mult)
            qn = qsb.tile([128, B], F32, tag="qn")
            nc.vector.tensor_scalar(out=qn[:], in0=qn1[:], scalar1=invtb[:, 0:1],
                                    scalar2=None, op0=ALU.mult)
            nc.sync.dma_start(out=d_qta[:, :], in_=qn[:, 0:128])
            nc.sync.dma_start(out=d_qtb[:, :], in_=qn[:, 128:256])

        nc.gpsimd.collective_compute(
            "AllGather", ALU.bypass, replica_groups=groups,
            ins=[d_qta.ap().opt()], outs=[d_qaa.ap().opt()])
        nc.gpsimd.collective_compute(
            "AllGather", ALU.bypass, replica_groups=groups,
            ins=[d_qtb.ap().opt()], outs=[d_qab.ap().opt()])

        # ---- Phase 3: keys (LN+l2norm folded into matmuls; sigma cancels) ----
        keysT = big.tile([128, COLS], F32)
        with tc.tile_pool(name="key_sb", bufs=3) as ksb, \
             tc.tile_pool(name="key_ps", bufs=2, space="PSUM") as kps:
            ysb = big.tile([128, COLS], F32)
            KCH = 512
            chunks = [(c0, min(COLS, c0 + KCH)) for c0 in range(0, COLS, KCH)]
            sqs = []
            for c0, c1 in chunks:
                w = c1 - c0
                py = kps.tile([128, KCH], F32, tag="y", space="PSUM")
                nc.tensor.matmul(out=py[:, :w], lhsT=wkt[:],
                                 rhs=hsave[:, c0:c1], start=True, stop=False)
                nc.tensor.matmul(out=py[:, :w], lhsT=uko[:],
                                 rhs=hsave[:, c0:c1], start=False, stop=True)
                nc.scalar.activation(ysb[:, c0:c1], py[:, :w], AF.Copy)
                y2 = ksb.tile([128, KCH], F32, tag="y2")
                nc.scalar.activation(y2[:, :w], py[:, :w], AF.Square)
                psq = kps.tile([128, KCH], F32, tag="s", space="PSUM")
                nc.tensor.matmul(out=psq[:, :w], lhsT=onesf[:],
                                 rhs=y2[:, :w], start=True, stop=True)
                sq = ksb.tile([128, KCH], F32, tag="sq")
                nc.scalar.sqrt(sq[:, :w], psq[:, :w])
                sqs.append((sq, c0, c1))
            for sq, c0, c1 in sqs:
                w = c1 - c0
                ri = ksb.tile([128, KCH], F32, tag="ri")
                nc.vector.reciprocal(ri[:, :w], sq[:, :w])
                nc.vector.tensor_tensor(out=keysT[:, c0:c1], in0=ysb[:, c0:c1],
                                        in1=ri[:, :w], op=ALU.mult)

        # ---- Phase 4: scores + local top-8 per target tile ----
        # qT block for tile T comes straight off the gathered buffer (already
        # feature-major).  PSUM is chunked 4x1024 (2 banks each) so the PE
        # streams while Act drains.
        v8 = big.tile([128, NT * 8], F32)
        i8 = big.tile([128, NT * 8], U32)
        i8f = big.tile([128, NT * 4], F32)
        selfm = big.tile([128, NT * 4], F32)

        def filter_and_exchange(par, d_cl_p, d_ca_p):
            # self-stock filter (self iff |idx - (lo+4.5)| < 5) on this
            # parity's tiles, then AllToAll their candidates
            v8v = v8[:].rearrange("p (T w) -> p T w", w=8)[:, par::2, :]
            i8v = i8[:].rearrange("p (T w) -> p T w", w=8)[:, par::2, :]
            hf = i8f[:].rearrange("p (T w) -> p T w", w=8)
            hm = selfm[:].rearrange("p (T w) -> p T w", w=8)
            nc.vector.tensor_copy(hf, i8v)
            slo_v = slo[:, par::2].rearrange("p T -> p T ()").to_broadcast(
                [128, NT // 2, 8])
            nc.vector.tensor_tensor(out=hf, in0=hf, in1=slo_v, op=ALU.subtract)
            nc.vector.tensor_tensor(out=hm, in0=hf, in1=hf, op=ALU.mult)
            nc.vector.tensor_scalar(out=selfm[:], in0=selfm[:], scalar1=25.0,
                                    scalar2=None, op0=ALU.is_lt)
            nc.vector.scalar_tensor_tensor(out=v8v, in0=hm, scalar=-1e30,
                                           in1=v8v, op0=ALU.mult, op1=ALU.add)
            cl_v = d_cl_p.ap().rearrange("(T p) w -> p T w", p=128)
            nc.sync.dma_start(out=cl_v[:, :, 0:8], in_=v8v.bitcast(U32))
            nc.sync.dma_start(out=cl_v[:, :, 8:16], in_=i8v)
            nc.gpsimd.collective_compute(
                "AllToAll", ALU.bypass, replica_groups=groups,
                ins=[d_cl_p.ap().opt()], outs=[d_ca_p.ap().opt()])

        with tc.tile_pool(name="sc_sb", bufs=2) as ssb_p, \
             tc.tile_pool(name="sc_q", bufs=3) as sqp, \
             tc.tile_pool(name="sc_ps", bufs=4, space="PSUM") as sps:
            for par, d_qx, d_cl_p, d_ca_p in ((0, d_qaa, d_cle, d_cae),
                                              (1, d_qab, d_clo, d_cao)):
                for T in range(par, NT, 2):
                    s = T // 2
                    qT = sqp.tile([128, 128], F32, tag="qT")
                    nc.sync.dma_start(out=qT[:], in_=d_qx[s * N:(s + 1) * N, :])
                    ssb = ssb_p.tile([128, COLS], F32, tag="ssb")
                    for c0 in range(0, COLS, CH):
                        c1 = min(COLS, c0 + CH)
                        ps = sps.tile([128, CH], F32, tag="sc", space="PSUM")
                        for m0 in range(c0, c1, 512):
                            m1 = min(c1, m0 + 512)
                            nc.tensor.matmul(out=ps[:, m0 - c0:m1 - c0], lhsT=qT[:],
                                             rhs=keysT[:, m0:m1], start=True, stop=True)
                        nc.scalar.activation(ssb[:, c0:c1], ps[:, 0:c1 - c0], AF.Copy)
                    nc.vector.max(out=v8[:, T * 8:(T + 1) * 8], in_=ssb[:])
                    nc.vector.max_index(out=i8[:, T * 8:(T + 1) * 8],
                                        in_max=v8[:, T * 8:(T + 1) * 8],
                                        in_values=ssb[:])
                filter_and_exchange(par, d_cl_p, d_ca_p)

        # ---- Phase 6: merge own 256 targets, z-gather, softmax, MLP ----
        # Candidate rows for local target i sit at {128*s + i} of the parity
        # exchange output: a regular strided DMA, no indirect gathers.
        # j=0 (even tiles) only needs the FIRST AllToAll, so its merge work
        # queues up underneath the odd-tile scores.
        with tc.tile_pool(name="m_sb", bufs=2) as msb, \
             tc.tile_pool(name="m_ps", bufs=1, space="PSUM") as mps:
            for j in range(2):
                ca_v = [d_cae, d_cao][j].ap().rearrange("(s i) w -> i s w", s=ND)
                mv = msb.tile([128, 128], U32, tag="mv")
                nc.sync.dma_start(
                    out=mv[:].rearrange("p (d w) -> p d w", w=16),
                    in_=ca_v[:, :, :])
                mvals = mv[:].bitcast(F32).rearrange(
                    "p (d w) -> p d w", w=16)[:, :, 0:8]
                midx = mv[:].rearrange("p (d w) -> p d w", w=16)[:, :, 8:16]
                mvalc = msb.tile([128, 64], F32, tag="mvalc")
                nc.vector.tensor_copy(mvalc[:], mvals)
                v8m = msb.tile([128, 8], F32, tag="v8m")
                nc.vector.max(out=v8m[:], in_=mvalc[:])
                pos8 = msb.tile([128, 8], U32, tag="pos8")
                nc.vector.max_index(out=pos8[:], in_max=v8m[:], in_values=mvalc[:])
                # gather global flat index by candidate position (one-hot trick)
                pos5f = msb.tile([128, 5], F32, tag="pos5f")
                nc.vector.tensor_copy(pos5f[:], pos8[:, 0:5])
                midxf = msb.tile([128, 64], F32, tag="midxf")
                nc.vector.tensor_copy(midxf[:], midx)
                nc.vector.tensor_tensor(out=midxf[:], in0=midxf[:],
                                        in1=base_f[:], op=ALU.add)
                eq = msb.tile([128, 5 * 64], F32, tag="eq")
                eq_v = eq[:].rearrange("p (k c) -> p k c", c=64)
                nc.vector.tensor_tensor(
                    out=eq_v,
                    in0=pos5f[:].rearrange("p k -> p k ()").to_broadcast([128, 5, 64]),
                    in1=iota_f[:].rearrange("p c -> p () c").to_broadcast([128, 5, 64]),
                    op=ALU.is_equal)
                nc.vector.tensor_tensor(
                    out=eq_v, in0=eq_v,
                    in1=midxf[:].rearrange("p c -> p () c").to_broadcast([128, 5, 64]),
                    op=ALU.mult)
                g5f = msb.tile([128, 5], F32, tag="g5f")
                nc.vector.tensor_reduce(out=g5f[:], in_=eq_v,
                                        axis=mybir.AxisListType.X, op=ALU.add)
                g5u = msb.tile([128, 5], U32, tag="g5u")
                nc.vector.tensor_copy(g5u[:], g5f[:])
                # z rows (6 raw feats + lag_bias + pad) for the 5 winners
                zb = msb.tile([128, 5 * 8], F32, tag="zb")
                for r in range(K):
                    nc.gpsimd.indirect_dma_start(
                        out=zb[:, r * 8:(r + 1) * 8], out_offset=None,
                        in_=d_xzb[:, :],
                        in_offset=IndirectOffsetOnAxis(ap=g5u[:, r:r + 1], axis=0))
                vb5 = msb.tile([128, 5], F32, tag="vb5")
                nc.vector.tensor_tensor(out=vb5[:], in0=v8m[:, 0:5],
                                        in1=zb[:, 6::8], op=ALU.add)
                # softmax over the 5 candidate scores
                mx = msb.tile([128, 1], F32, tag="mx")
                nc.vector.tensor_reduce(out=mx[:], in_=vb5[:],
                                        axis=mybir.AxisListType.X, op=ALU.max)
                nmx = msb.tile([128, 1], F32, tag="nmx")
                nc.vector.tensor_scalar(out=nmx[:], in0=mx[:],
                                        scalar1=-1.0, scalar2=None, op0=ALU.mult)
                e5 = msb.tile([128, 5], F32, tag="e5")
                nc.scalar.activation(e5[:], vb5[:], AF.Exp, bias=nmx[:, 0:1])
                ssum = msb.tile([128, 1], F32, tag="ssum")
                nc.vector.tensor_reduce(out=ssum[:], in_=e5[:],
                                        axis=mybir.AxisListType.X, op=ALU.add)
                rs = msb.tile([128, 1], F32, tag="rs")
                nc.vector.reciprocal(rs[:], ssum[:])
                w5 = msb.tile([128, 5], F32, tag="w5")
                nc.vector.tensor_scalar(out=w5[:], in0=e5[:],
                                        scalar1=rs[:, 0:1], scalar2=None, op0=ALU.mult)
                # z_agg = sum_r w_r * z_r ; feat = [z_agg, z_0]
                wz = msb.tile([128, 5 * 6], F32, tag="wz")
                zview = zb[:].rearrange("p (r w) -> p r w", w=8)[:, :, 0:6]
                nc.vector.tensor_tensor(
                    out=wz[:].rearrange("p (r f) -> p r f", f=6),
                    in0=zview,
                    in1=w5[:].rearrange("p r -> p r ()").to_broadcast([128, 5, 6]),
                    op=ALU.mult)
                feat = msb.tile([128, 2 * F], F32, tag="feat")
                nc.vector.tensor_reduce(
                    out=feat[:, 0:6],
                    in_=wz[:].rearrange("p (r f) -> p f r", f=6),
                    axis=mybir.AxisListType.X, op=ALU.add)
                nc.vector.tensor_copy(feat[:, 6:12], zb[:, 0:6])
                # MLP head (tiny fp32 matmuls)
                pft = mps.tile([2 * F, 128], F32, tag="pft", space="PSUM")
                nc.tensor.transpose(out=pft[0:2 * F, 0:128], in_=feat[:, :],
                                    identity=ident[:])
                featT = msb.tile([2 * F, 128], F32, tag="featT")
                nc.scalar.activation(featT[:], pft[0:2 * F, 0:128], AF.Copy)
                ph1 = mps.tile([64, 128], F32, tag="ph1", space="PSUM")
                nc.tensor.matmul(out=ph1[:], lhsT=w1t[:], rhs=featT[:], start=True, stop=True)
                h1 = msb.tile([64, 128], F32, tag="h1")
                nc.scalar.activation(h1[:], ph1[:], AF.Relu, bias=b1c[:, 0:1])
                ph2 = mps.tile([32, 128], F32, tag="ph2", space="PSUM")
                nc.tensor.matmul(out=ph2[:], lhsT=w2t[:], rhs=h1[:], start=True, stop=True)
                h2 = msb.tile([32, 128], F32, tag="h2")
                nc.scalar.activation(h2[:], ph2[:], AF.Relu, bias=b2c[:, 0:1])
                py_ = mps.tile([1, 128], F32, tag="py", space="PSUM")
                nc.tensor.matmul(out=py_[:], lhsT=w3t[:], rhs=h2[:], start=True, stop=True)
                yrow = msb.tile([1, 128], F32, tag="yrow")
                nc.scalar.activation(yrow[:], py_[:], AF.Identity, bias=b3c[0:1, 0:1])
                pyt = mps.tile([128, 1], F32, tag="pyt", space="PSUM")
                nc.tensor.transpose(out=pyt[:, 0:1], in_=yrow[0:1, :],
                                    identity=ident[0:1, 0:1])
                ycol = msb.tile([128, 1], F32, tag="ycol")
                nc.vector.tensor_copy(ycol[:], pyt[:, 0:1])
                nc.sync.dma_start(out=d_y[j * 128:(j + 1) * 128, :], in_=ycol[:])

        big.release()
        cpool.release()

    nc.compile()
    return nc


_CACHED_NC = None


def _prep_inputs(X_scaled, X_raw, target_idx, lstm_Wih, lstm_Whh, lstm_bih,
                 lstm_bhh, ln_g, ln_b, WQ, WK, log_temp, lag_bias,
                 W1, b1, W2, b2, W3, b3):
    f32 = np.float32
    assert np.all(np.asarray(ln_b) == 0.0), "kernel assumes ln_b == 0"
    tix = np.asarray(target_idx).astype(np.int64)

    # distinct targets grouped by owner core; local stock order permuted so
    # the distinct-target stocks occupy slots 0..cnt-1
    dist = np.unique(tix)                          # sorted -> grouped by core
    perm = np.zeros((ND, SS), np.int64)            # slot -> local stock id
    dist_slot = np.full(S, -1, np.int64)           # stock -> global dist slot
    for dd in range(ND):
        loc = dist[(dist >= dd * SS) & (dist < (dd + 1) * SS)] - dd * SS
        cnt = loc.size
        assert cnt <= B, f"core {dd} has {cnt} distinct targets (> {B})"
        rest = np.setdiff1d(np.arange(SS), loc)
        perm[dd] = np.concatenate([loc, rest])
        dist_slot[dd * SS + loc] = dd * B + np.arange(cnt)
    tgt_slot = dist_slot[tix]                      # target -> global dist slot
    assert np.all(tgt_slot >= 0)

    bias = (np.asarray(lstm_bih) + np.asarray(lstm_bhh)).astype(f32)
    gperm = np.r_[0:N, N:2 * N, 3 * N:4 * N, 2 * N:3 * N]    # [i, f, o, g]
    g_ln = np.asarray(ln_g).astype(f32)
    wq_f = (np.asarray(WQ) * g_ln[None, :]).astype(f32)
    wk_f = (np.asarray(WK) * g_ln[None, :]).astype(f32)
    uq = np.asarray(WQ) @ g_ln
    uk = np.asarray(WK) @ g_ln
    inv_temp = np.asarray(
        1.0 / np.clip(np.exp(np.asarray(log_temp, np.float64)), 0.1, np.sqrt(N)),
        f32).reshape(1, 1)

    # XZB table: flat (core, local col) -> [6 raw feats at lag_pos, lag_bias, 0]
    Xr = np.asarray(X_raw)[0].astype(f32)                    # [S, L, F]
    lb = np.asarray(lag_bias).astype(f32)
    lagpos = np.clip(L - 1 - (LMAX - np.arange(LMAX)), 0, L - 1)
    stock_of_col = (np.arange(S * LMAX) // LMAX)             # flat -> perm slot
    core_of_col = stock_of_col // SS
    stock_perm = (perm + (np.arange(ND) * SS)[:, None]).reshape(-1)  # global
    xzb = np.zeros((S * LMAX, 8), f32)
    xzb[:, 0:6] = Xr[stock_perm[stock_of_col], lagpos[np.arange(S * LMAX) % LMAX]]
    xzb[:, 6] = np.tile(lb, S)

    # self-column center per (row p, tile T): tiles 2d, 2d+1 hold core d's
    # targets; on core d the target's own stock sits at score column
    # 10*slot + 4.5 (slot == dist slot by construction)
    selflo = np.full((128, NT, ND), -1e9, f32)
    for dd in range(ND):
        for j in range(2):
            sl = np.arange(128) + 128 * j
            selflo[:, 2 * dd + j, dd] = sl * LMAX + 4.5

    import ml_dtypes
    bf = ml_dtypes.bfloat16
    Xs = np.asarray(X_scaled)[0].astype(f32)                 # [S, L, F]
    # fp32 weights fold the exact bias via xt's ones-row; the bf16 weights
    # carry the bias as a bf16 (hi, lo) pair over TWO ones-rows, since a
    # single bf16-rounded bias is enough noise to flip a top-5 selection
    wih_t = np.ascontiguousarray(np.vstack([
        np.asarray(lstm_Wih).astype(f32).T, bias[None, :]])[:, gperm])
    whh_t = np.ascontiguousarray(np.asarray(lstm_Whh).astype(f32).T[:, gperm])
    bias_hi = bias[gperm].astype(bf)
    bias_lo = (bias[gperm] - bias_hi.astype(f32)).astype(bf)
    wihb_t = np.vstack([wih_t[:F].astype(bf), bias_hi[None, :], bias_lo[None, :]])
    whhb_t = whh_t.astype(bf)
    # g-gate columns x2 (exact in bf16): early steps compute tanh(g) as
    # 2*sigmoid(2g) - 1 so o and g share one packed sigmoid
    wihb_t[:, 3 * N:] = wihb_t[:, 3 * N:] * bf(2.0)
    whhb_t[:, 3 * N:] = whhb_t[:, 3 * N:] * bf(2.0)
    common = dict(
        wih_t=wih_t, whh_t=whh_t,
        wihb_t=np.ascontiguousarray(wihb_t), whhb_t=np.ascontiguousarray(whhb_t),
        wq_t=np.ascontiguousarray(wq_f.T), wk_t=np.ascontiguousarray(wk_f.T),
        negu_q=np.ascontiguousarray((-uq.astype(f32) / N).reshape(1, N)),
        negu_k=np.ascontiguousarray((-uk.astype(f32) / N).reshape(1, N)),
        invt=inv_temp, xzb=xzb,
        w1_t=np.ascontiguousarray(np.asarray(W1).astype(f32).T),
        w2_t=np.ascontiguousarray(np.asarray(W2).astype(f32).T),
        w3_t=np.ascontiguousarray(np.asarray(W3).astype(f32).T),
        b1c=np.asarray(b1).astype(f32).reshape(64, 1),
        b2c=np.asarray(b2).astype(f32).reshape(32, 1),
        b3c=np.asarray(b3).astype(f32).reshape(1, 1),
    )
    in_maps = []
    for d in range(ND):
        Xd = Xs[d * SS:(d + 1) * SS][perm[d]]                # [SS, L, F] permuted
        xtv = np.vstack([Xd.transpose(2, 1, 0).reshape(F, L * SS),
                         np.ones((1, SS * L), f32)])
        xtb_d = np.vstack([xtv[:F, :T0 * SS], np.ones((2, T0 * SS), f32)])
        in_maps.append(dict(
            common,
            xtb=np.ascontiguousarray(xtb_d).astype(bf),
            xt=np.ascontiguousarray(xtv[:, T0 * SS:]),
            selflo=np.ascontiguousarray(selflo[:, :, d]),
        ))
    return in_maps, tgt_slot


def kernel(**inputs):
    global _CACHED_NC
    if _CACHED_NC is None:
        _CACHED_NC = build_program()
    nc = _CACHED_NC
    in_maps, tgt_slot = _prep_inputs(**inputs)
    res = run_bass_kernel_spmd(nc, in_maps, core_ids=list(range(ND)))
    ydist = np.concatenate([res.results[d]["y"][:, 0] for d in range(ND)])
    return ydist[tgt_slot].astype(np.float32)


# revision 42
# speedup vs baseline: 1.0100x; 1.0100x over previous
"""Trainium2 Bass kernel for nn_DeltaLag (LSTM encoder + lagged cross-attention
top-k + MLP head), distributed over 8 NeuronCores.

Key structure (v2):
- Stocks are split 375/core; each core's local stock order is PERMUTED so the
  distinct-target stocks come first (<=256 per core, asserted).  That makes
  the query exchange transpose-free and the merge block-contiguous.
- target_idx has repeats: only ~1904 of 3000 targets are distinct.  Scores /
  top-k / merge run over 2048 padded distinct targets (16 tiles); the final
  [3000] output is a host-side gather.
- LSTM: first T0=20 steps use bf16 matmuls (errors decay ~0.5/step through
  the forget gates; validated zero top-5 flips at bf16-level noise), last 20
  steps full fp32, A/B stock-half interleaved to hide the per-step serial
  chain.  Scores/keys/queries stay fp32: the #5-#6 score gap is as small as
  7e-8, so any score noise above ~1e-7 flips selections.
- Candidate exchange is an AllToAll (each core only receives its own 256
  targets' candidates), q exchange is an AllGather of [128,128] fp32 blocks;
  both are split even/odd so half of each overlaps the score computation.
"""

import sys

sys.path.insert(0, "/opt/trn_rl_repo")

import numpy as np

import concourse.bacc as bacc
import concourse.mybir as mybir
import concourse.tile as tile
from concourse.bass import IndirectOffsetOnAxis
from concourse.bass_utils import run_bass_kernel_spmd
from concourse.masks import make_identity

F32 = mybir.dt.float32
BF16 = mybir.dt.bfloat16
U32 = mybir.dt.uint32
AF = mybir.ActivationFunctionType
ALU = mybir.AluOpType

S, F, N, L, LMAX, K = 3000, 6, 128, 40, 10, 5
ND = 8                      # cores
SS = S // ND                # stocks per core
COLS = SS * LMAX            # score columns per core
B = 256                     # distinct-target slots per core
NPOS = ND * B               # padded distinct targets (2048)
NT = NPOS // 128            # target tiles (16)
T0 = 20                     # LSTM steps run with bf16 matmuls
CH = 1024                   # score PSUM chunk (2 banks)


def build_program():
    nc = bacc.Bacc("TRN2", target_bir_lowering=False, debug=False,
                   enable_asserts=True, num_devices=ND)

    # ---- I/O ----
    # xt is split: steps < T0 arrive pre-cast to bf16 (cheap matmuls), the
    # rest in fp32.  The bf16 side carries TWO ones-rows so the bias can be
    # folded as bias_hi + bias_lo (bf16 pair, ~1e-6 accurate; the residual
    # decays through the forget gates).  The fp32 side folds bias exactly.
    d_xtb = nc.dram_tensor("xtb", [F + 2, SS * T0], BF16, kind="ExternalInput")
    d_xt = nc.dram_tensor("xt", [F + 1, SS * (L - T0)], F32, kind="ExternalInput")
    d_wihb = nc.dram_tensor("wihb_t", [F + 2, 4 * N], BF16, kind="ExternalInput")
    d_whhb = nc.dram_tensor("whhb_t", [N, 4 * N], BF16, kind="ExternalInput")
    d_wih = nc.dram_tensor("wih_t", [F + 1, 4 * N], F32, kind="ExternalInput")
    d_whh = nc.dram_tensor("whh_t", [N, 4 * N], F32, kind="ExternalInput")
    d_wqt = nc.dram_tensor("wq_t", [N, N], F32, kind="ExternalInput")
    d_wkt = nc.dram_tensor("wk_t", [N, N], F32, kind="ExternalInput")
    d_nuq = nc.dram_tensor("negu_q", [1, N], F32, kind="ExternalInput")
    d_nuk = nc.dram_tensor("negu_k", [1, N], F32, kind="ExternalInput")
    d_invt = nc.dram_tensor("invt", [1, 1], F32, kind="ExternalInput")
    d_slo = nc.dram_tensor("selflo", [128, NT], F32, kind="ExternalInput")
    d_xzb = nc.dram_tensor("xzb", [S * LMAX, 8], F32, kind="ExternalInput")
    d_w1t = nc.dram_tensor("w1_t", [2 * F, 64], F32, kind="ExternalInput")
    d_w2t = nc.dram_tensor("w2_t", [64, 32], F32, kind="ExternalInput")
    d_w3t = nc.dram_tensor("w3_t", [32, 1], F32, kind="ExternalInput")
    d_b1 = nc.dram_tensor("b1c", [64, 1], F32, kind="ExternalInput")
    d_b2 = nc.dram_tensor("b2c", [32, 1], F32, kind="ExternalInput")
    d_b3 = nc.dram_tensor("b3c", [1, 1], F32, kind="ExternalInput")

    d_y = nc.dram_tensor("y", [B, 1], F32, kind="ExternalOutput")

    # q exchange and candidate exchange are split even/odd so the second half
    # of each overlaps compute: even-tile scores start after the first (half
    # size) AllGather; even-tile candidates exchange + merge run underneath
    # the odd-tile scores.
    d_qta = nc.dram_tensor("qbl_ta", [N, 128], F32)
    d_qtb = nc.dram_tensor("qbl_tb", [N, 128], F32)
    d_qaa = nc.dram_tensor("qba_ta", [ND * N, 128], F32, addr_space="Shared")
    d_qab = nc.dram_tensor("qba_tb", [ND * N, 128], F32, addr_space="Shared")
    d_cle = nc.dram_tensor("cand_le", [ND * 128, 16], U32)
    d_clo = nc.dram_tensor("cand_lo", [ND * 128, 16], U32)
    d_cae = nc.dram_tensor("cand_ae", [ND * 128, 16], U32)
    d_cao = nc.dram_tensor("cand_ao", [ND * 128, 16], U32)

    groups = [list(range(ND))]

    with tile.TileContext(nc) as tc:
        cpool = tc.alloc_tile_pool(name="const", bufs=1)
        big = tc.alloc_tile_pool(name="big", bufs=1)

        # ---- constants / params to SBUF ----
        ident = cpool.tile([128, 128], F32)
        make_identity(nc, ident[:])
        ones1 = cpool.tile([1, 128], F32)
        nc.vector.memset(ones1[:], 1.0)
        onesf = cpool.tile([128, 128], F32)
        nc.vector.memset(onesf[:], 1.0)

        def load(pool, dram, shape, dtype=F32):
            t = pool.tile(shape, dtype, tag=f"ld_{dram.name}")
            nc.sync.dma_start(out=t[:], in_=dram[:, :])
            return t

        xtb = big.tile([F + 2, SS * T0], BF16)
        xt = big.tile([F + 1, SS * (L - T0)], F32)
        wih = load(cpool, d_wih, [F + 1, 4 * N])
        whh = load(cpool, d_whh, [N, 4 * N])
        wihb = load(cpool, d_wihb, [F + 2, 4 * N], BF16)
        whhb = load(cpool, d_whhb, [N, 4 * N], BF16)
        wqt = load(cpool, d_wqt, [N, N])
        wkt = load(cpool, d_wkt, [N, N])
        nuq = load(cpool, d_nuq, [1, N])
        nuk = load(cpool, d_nuk, [1, N])
        invt = load(cpool, d_invt, [1, 1])
        slo = load(cpool, d_slo, [128, NT])
        w1t = load(cpool, d_w1t, [2 * F, 64])
        w2t = load(cpool, d_w2t, [64, 32])
        w3t = load(cpool, d_w3t, [32, 1])
        b1c = load(cpool, d_b1, [64, 1])
        b2c = load(cpool, d_b2, [32, 1])
        b3c = load(cpool, d_b3, [1, 1])
        for c in range(4):
            nc.sync.dma_start(out=xtb[:, c * 5 * SS:(c + 1) * 5 * SS],
                              in_=d_xtb[:, c * 5 * SS:(c + 1) * 5 * SS])
        for c in range(4):
            nc.sync.dma_start(out=xt[:, c * 5 * SS:(c + 1) * 5 * SS],
                              in_=d_xt[:, c * 5 * SS:(c + 1) * 5 * SS])

        invtb = cpool.tile([128, 1], F32)
        nc.gpsimd.partition_broadcast(invtb[:], invt[:], channels=128)

        iota_u = cpool.tile([128, 64], U32)
        nc.gpsimd.iota(iota_u[:], pattern=[[1, 64]], base=0, channel_multiplier=0)
        iota_f = cpool.tile([128, 64], F32)
        nc.vector.tensor_copy(iota_f[:], iota_u[:])
        base_u = cpool.tile([128, 64], U32)
        nc.gpsimd.iota(base_u[:], pattern=[[COLS, 8], [0, 8]], base=0,
                       channel_multiplier=0)
        base_f = cpool.tile([128, 64], F32)
        nc.vector.tensor_copy(base_f[:], base_u[:])

        # rank-1 LN-fold correction matrices: rows n, cols p -> -u[p]/128
        with tc.tile_pool(name="ppre", bufs=1, space="PSUM") as ppre:
            uqo = cpool.tile([128, 128], F32)
            uko = cpool.tile([128, 128], F32)
            pq = ppre.tile([128, 128], F32, space="PSUM")
            nc.tensor.matmul(out=pq[:], lhsT=ones1[:], rhs=nuq[:], start=True, stop=True)
            nc.scalar.activation(uqo[:], pq[:], AF.Copy)
            pk = ppre.tile([128, 128], F32, space="PSUM")
            nc.tensor.matmul(out=pk[:], lhsT=ones1[:], rhs=nuk[:], start=True, stop=True)
            nc.scalar.activation(uko[:], pk[:], AF.Copy)

        # ---- Phase 1: LSTM over the 375 local stocks ----
        # h,c layout [n=128, s]; last-10 hidden states land in hsave[n, s*10+k].
        # Gate columns in wih/whh are host-permuted to [i, f, o, g]; the bias
        # is folded into the xproj matmul via xt's constant-1 row.
        # Steps < T0 run the matmuls in fp32r (error decays through the forget
        # gates); the last steps are full fp32.
        # Two independent stock-halves (A: cols 0:188, B: 188:375) pipeline so
        # the per-step serial chain of one half hides under the other's
        # engine work.  All 4 gate pre-activations of a half live in ONE
        # 2-bank PSUM tile (i,f,o at 188-col slices, g at 564+), so i/f/o
        # share a single packed Sigmoid.
        GW = 188
        halves = [(0, GW), (GW, SS - GW)]
        hsave = big.tile([128, COLS], F32)
        with tc.tile_pool(name="lstm_sb", bufs=2) as lsb, \
             tc.tile_pool(name="lstm_c", bufs=2) as lcp, \
             tc.tile_pool(name="lstm_ps", bufs=2, space="PSUM") as lps:
            h_prev = []
            c_prev = []
            for hh, (c0, w) in enumerate(halves):
                h0 = lsb.tile([128, GW], BF16, tag=f"h0_{hh}")
                nc.vector.memset(h0[:, :w], 0.0)
                c0t = lcp.tile([128, GW], F32, tag=f"c_{hh}")
                nc.vector.memset(c0t[:, :w], 0.0)
                h_prev.append(h0[:, :w])
                c_prev.append(c0t)
            for t in range(L):
                early = t < T0
                wih_t = wihb if early else wih
                whh_t = whhb if early else whh
                for hh, (c0, w) in enumerate(halves):
                    xt_t = (xtb[:, t * SS + c0:t * SS + c0 + w] if early
                            else xt[:, (t - T0) * SS + c0:(t - T0) * SS + c0 + w])
                    P1 = lps.tile([128, 512], F32, tag=f"gif{hh}", space="PSUM")
                    P2 = lps.tile([128, 512], F32, tag=f"gog{hh}", space="PSUM")
                    slots = [(P1, 0), (P1, w), (P2, 0), (P2, w)]  # i, f, o, g
                    for g in range(4):
                        Pg, o0 = slots[g]
                        nc.tensor.matmul(out=Pg[:, o0:o0 + w],
                                         lhsT=wih_t[:, g * N:(g + 1) * N],
                                         rhs=xt_t, start=True, stop=False)
                        nc.tensor.matmul(out=Pg[:, o0:o0 + w],
                                         lhsT=whh_t[:, g * N:(g + 1) * N],
                                         rhs=h_prev[hh], start=False, stop=True)
                    sifo = lsb.tile([128, 3 * GW], F32, tag=f"sifo{hh}")
                    nc.scalar.activation(sifo[:, :2 * w], P1[:, 0:2 * w], AF.Sigmoid)
                    si = sifo[:, 0:w]
                    sf = sifo[:, w:2 * w]
                    t1 = lsb.tile([128, GW], F32, tag=f"t1{hh}")
                    if early:
                        # g-gate weights host-scaled x2: one packed sigmoid
                        # yields [so, sg] with tanh(g) = 2*sg - 1, so
                        # t1 = si*tanh(g) = 2*(si*sg) - si (on Pool, off the
                        # DVE chain)
                        sog = lsb.tile([128, 2 * GW], F32, tag=f"sog{hh}")
                        nc.scalar.activation(sog[:, :2 * w], P2[:, 0:2 * w],
                                             AF.Sigmoid)
                        so = sog[:, 0:w]
                        # tanh(g) = 2*sg - 1 with immediate scalars (the Ptr
                        # variant of TensorScalar is not a legal Pool opcode)
                        u = lsb.tile([128, GW], F32, tag=f"u{hh}")
                        nc.gpsimd.tensor_scalar(out=u[:, :w], in0=sog[:, w:2 * w],
                                                scalar1=2.0, scalar2=-1.0,
                                                op0=ALU.mult, op1=ALU.add)
                        nc.gpsimd.tensor_tensor(out=t1[:, :w], in0=u[:, :w],
                                                in1=si, op=ALU.mult)
                    else:
                        nc.scalar.activation(sifo[:, 2 * w:3 * w], P2[:, 0:w],
                                             AF.Sigmoid)
                        tg = lsb.tile([128, GW], F32, tag=f"tg{hh}")
                        nc.scalar.activation(tg[:, :w], P2[:, w:2 * w], AF.Tanh)
                        so = sifo[:, 2 * w:3 * w]
                        nc.vector.tensor_tensor(out=t1[:, :w], in0=si,
                                                in1=tg[:, :w], op=ALU.mult)
                    c2 = lsb.tile([128, GW], F32, tag=f"c2{hh}")
                    nc.gpsimd.tensor_tensor(out=c2[:, :w], in0=sf,
                                            in1=c_prev[hh][:, :w], op=ALU.mult)
                    c_new = lcp.tile([128, GW], F32, tag=f"c_{hh}")
                    nc.vector.tensor_tensor(out=c_new[:, :w], in0=c2[:, :w],
                                            in1=t1[:, :w], op=ALU.add)
                    th = lsb.tile([128, GW], F32, tag=f"th{hh}")
                    nc.scalar.activation(th[:, :w], c_new[:, :w], AF.Tanh)
                    if t >= L - LMAX:
                        k = t - (L - LMAX)
                        h_out = hsave[:, k + LMAX * c0:
                                      k + LMAX * (c0 + w - 1) + 1:LMAX]
                        nc.gpsimd.tensor_tensor(out=h_out, in0=so, in1=th[:, :w],
                                                op=ALU.mult)
                        h_prev[hh] = h_out
                    else:
                        hdt = BF16 if (t + 1 < T0) else F32
                        hn = lsb.tile([128, GW], hdt,
                                      tag=f"hn{hh}" if hdt == F32 else f"hnb{hh}")
                        nc.gpsimd.tensor_tensor(out=hn[:, :w], in0=so,
                                                in1=th[:, :w], op=ALU.mult)
                        h_prev[hh] = hn[:, :w]
                    c_prev[hh] = c_new

        # ---- Phase 2: queries for the local distinct-target slots ----
        # Local stock order is host-permuted so slots 0..B-1 are the distinct
        # targets owned by this core; q comes out feature-major, so the
        # exchange needs no transposes at all.
        with tc.tile_pool(name="q_sb", bufs=2) as qsb, \
             tc.tile_pool(name="q_ps", bufs=2, space="PSUM") as qps:
            h39 = hsave[:, (LMAX - 1):(B * LMAX):LMAX]
            pyq = qps.tile([128, B], F32, tag="yq", space="PSUM")
            nc.tensor.matmul(out=pyq[:], lhsT=wqt[:], rhs=h39, start=True, stop=False)
            nc.tensor.matmul(out=pyq[:], lhsT=uqo[:], rhs=h39, start=False, stop=True)
            yq = qsb.tile([128, B], F32, tag="yq_sb")
            nc.scalar.activation(yq[:], pyq[:], AF.Copy)
            y2q = qsb.tile([128, B], F32, tag="y2q")
            nc.scalar.activation(y2q[:], pyq[:], AF.Square)
            psq = qps.tile([128, B], F32, tag="sq", space="PSUM")
            nc.tensor.matmul(out=psq[:], lhsT=onesf[:], rhs=y2q[:], start=True, stop=True)
            sq = qsb.tile([128, B], F32, tag="sqq")
            nc.scalar.sqrt(sq[:], psq[:])
            ri = qsb.tile([128, B], F32, tag="riq")
            nc.vector.reciprocal(ri[:], sq[:])
            qn1 = qsb.tile([128, B], F32, tag="qn1")
            nc.vector.tensor_tensor(out=qn1[:], in0=yq[:], in1=ri[:], op=ALU.mult)
            qn = qsb.tile([128, B], F32, tag="qn")
            nc.vector.tensor_scalar(out=qn[:], in0=qn1[:], scalar1=invtb[:, 0:1],
                                    scalar2=None, op0=ALU.mult)
            nc.sync.dma_start(out=d_qta[:, :], in_=qn[:, 0:128])
            nc.sync.dma_start(out=d_qtb[:, :], in_=qn[:, 128:256])

        nc.gpsimd.collective_compute(
            "AllGather", ALU.bypass, replica_groups=groups,
            ins=[d_qta.ap().opt()], outs=[d_qaa.ap().opt()])
        nc.gpsimd.collective_compute(
            "AllGather", ALU.bypass, replica_groups=groups,
            ins=[d_qtb.ap().opt()], outs=[d_qab.ap().opt()])

        # ---- Phase 3: keys (LN+l2norm folded into matmuls; sigma cancels) ----
        keysT = big.tile([128, COLS], F32)
        with tc.tile_pool(name="key_sb", bufs=3) as ksb, \
             tc.tile_pool(name="key_ps", bufs=2, space="PSUM") as kps:
            ysb = big.tile([128, COLS], F32)
            KCH = 512
            chunks = [(c0, min(COLS, c0 + KCH)) for c0 in range(0, COLS, KCH)]
            sqs = []
            for c0, c1 in chunks:
                w = c1 - c0
                py = kps.tile([128, KCH], F32, tag="y", space="PSUM")
                nc.tensor.matmul(out=py[:, :w], lhsT=wkt[:],
                                 rhs=hsave[:, c0:c1], start=True, stop=False)
                nc.tensor.matmul(out=py[:, :w], lhsT=uko[:],
                                 rhs=hsave[:, c0:c1], start=False, stop=True)
                nc.scalar.activation(ysb[:, c0:c1], py[:, :w], AF.Copy)
                y2 = ksb.tile([128, KCH], F32, tag="y2")
                nc.scalar.activation(y2[:, :w], py[:, :w], AF.Square)
                psq = kps.tile([128, KCH], F32, tag="s", space="PSUM")
                nc.tensor.matmul(out=psq[:, :w], lhsT=onesf[:],
                                 rhs=y2[:, :w], start=True, stop=True)
                sq = ksb.tile([128, KCH], F32, tag="sq")
                nc.scalar.sqrt(sq[:, :w], psq[:, :w])
                sqs.append((sq, c0, c1))
            for sq, c0, c1 in sqs:
                w = c1 - c0
                ri = ksb.tile([128, KCH], F32, tag="ri")
                nc.vector.reciprocal(ri[:, :w], sq[:, :w])
                nc.vector.tensor_tensor(out=keysT[:, c0:c1], in0=ysb[:, c0:c1],
                                        in1=ri[:, :w], op=ALU.mult)

        # ---- Phase 4: scores + local top-8 per target tile ----
        # qT block for tile T comes straight off the gathered buffer (already
        # feature-major).  PSUM is chunked 4x1024 (2 banks each) so the PE
        # streams while Act drains.
        v8 = big.tile([128, NT * 8], F32)
        i8 = big.tile([128, NT * 8], U32)
        i8f = big.tile([128, NT * 4], F32)
        selfm = big.tile([128, NT * 4], F32)

        def filter_and_exchange(par, d_cl_p, d_ca_p):
            # self-stock filter (self iff |idx - (lo+4.5)| < 5) on this
            # parity's tiles, then AllToAll their candidates
            v8v = v8[:].rearrange("p (T w) -> p T w", w=8)[:, par::2, :]
            i8v = i8[:].rearrange("p (T w) -> p T w", w=8)[:, par::2, :]
            hf = i8f[:].rearrange("p (T w) -> p T w", w=8)
            hm = selfm[:].rearrange("p (T w) -> p T w", w=8)
            nc.vector.tensor_copy(hf, i8v)
            slo_v = slo[:, par::2].rearrange("p T -> p T ()").to_broadcast(
                [128, NT // 2, 8])
            nc.vector.tensor_tensor(out=hf, in0=hf, in1=slo_v, op=ALU.subtract)
            nc.vector.tensor_tensor(out=hm, in0=hf, in1=hf, op=ALU.mult)
            nc.vector.tensor_scalar(out=selfm[:], in0=selfm[:], scalar1=25.0,
                                    scalar2=None, op0=ALU.is_lt)
            nc.vector.scalar_tensor_tensor(out=v8v, in0=hm, scalar=-1e30,
                                           in1=v8v, op0=ALU.mult, op1=ALU.add)
            cl_v = d_cl_p.ap().rearrange("(T p) w -> p T w", p=128)
            nc.sync.dma_start(out=cl_v[:, :, 0:8], in_=v8v.bitcast(U32))
            nc.sync.dma_start(out=cl_v[:, :, 8:16], in_=i8v)
            nc.gpsimd.collective_compute(
                "AllToAll", ALU.bypass, replica_groups=groups,
                ins=[d_cl_p.ap().opt()], outs=[d_ca_p.ap().opt()])

        with tc.tile_pool(name="sc_sb", bufs=2) as ssb_p, \
             tc.tile_pool(name="sc_q", bufs=3) as sqp, \
             tc.tile_pool(name="sc_ps", bufs=4, space="PSUM") as sps:
            for par, d_qx, d_cl_p, d_ca_p in ((0, d_qaa, d_cle, d_cae),
                                              (1, d_qab, d_clo, d_cao)):
                for T in range(par, NT, 2):
                    s = T // 2
                    qT = sqp.tile([128, 128], F32, tag="qT")
                    nc.sync.dma_start(out=qT[:], in_=d_qx[s * N:(s + 1) * N, :])
                    ssb = ssb_p.tile([128, COLS], F32, tag="ssb")
                    for c0 in range(0, COLS, CH):
                        c1 = min(COLS, c0 + CH)
                        ps = sps.tile([128, CH], F32, tag="sc", space="PSUM")
                        for m0 in range(c0, c1, 512):
                            m1 = min(c1, m0 + 512)
                            nc.tensor.matmul(out=ps[:, m0 - c0:m1 - c0], lhsT=qT[:],
                                             rhs=keysT[:, m0:m1], start=True, stop=True)
                        nc.scalar.activation(ssb[:, c0:c1], ps[:, 0:c1 - c0], AF.Copy)
                    nc.vector.max(out=v8[:, T * 8:(T + 1) * 8], in_=ssb[:])
                    nc.vector.max_index(out=i8[:, T * 8:(T + 1) * 8],
                                        in_max=v8[:, T * 8:(T + 1) * 8],
                                        in_values=ssb[:])
                filter_and_exchange(par, d_cl_p, d_ca_p)

        # ---- Phase 6: merge own 256 targets, z-gather, softmax, MLP ----
        # Candidate rows for local target i sit at {128*s + i} of the parity
        # exchange output: a regular strided DMA, no indirect gathers.
        # j=0 (even tiles) only needs the FIRST AllToAll, so its merge work
        # queues up underneath the odd-tile scores.
        with tc.tile_pool(name="m_sb", bufs=2) as msb, \
             tc.tile_pool(name="m_ps", bufs=1, space="PSUM") as mps:
            for j in range(2):
                ca_v = [d_cae, d_cao][j].ap().rearrange("(s i) w -> i s w", s=ND)
                mv = msb.tile([128, 128], U32, tag="mv")
                nc.sync.dma_start(
                    out=mv[:].rearrange("p (d w) -> p d w", w=16),
                    in_=ca_v[:, :, :])
                mvals = mv[:].bitcast(F32).rearrange(
                    "p (d w) -> p d w", w=16)[:, :, 0:8]
                midx = mv[:].rearrange("p (d w) -> p d w", w=16)[:, :, 8:16]
                mvalc = msb.tile([128, 64], F32, tag="mvalc")
                nc.vector.tensor_copy(mvalc[:], mvals)
                v8m = msb.tile([128, 8], F32, tag="v8m")
                nc.vector.max(out=v8m[:], in_=mvalc[:])
                pos8 = msb.tile([128, 8], U32, tag="pos8")
                nc.vector.max_index(out=pos8[:], in_max=v8m[:], in_values=mvalc[:])
                # gather global flat index by candidate position (one-hot trick)
                pos5f = msb.tile([128, 5], F32, tag="pos5f")
                nc.vector.tensor_copy(pos5f[:], pos8[:, 0:5])
                midxf = msb.tile([128, 64], F32, tag="midxf")
                nc.vector.tensor_copy(midxf[:], midx)
                nc.vector.tensor_tensor(out=midxf[:], in0=midxf[:],
                                        in1=base_f[:], op=ALU.add)
                eq = msb.tile([128, 5 * 64], F32, tag="eq")
                eq_v = eq[:].rearrange("p (k c) -> p k c", c=64)
                nc.vector.tensor_tensor(
                    out=eq_v,
                    in0=pos5f[:].rearrange("p k -> p k ()").to_broadcast([128, 5, 64]),
                    in1=iota_f[:].rearrange("p c -> p () c").to_broadcast([128, 5, 64]),
                    op=ALU.is_equal)
                nc.vector.tensor_tensor(
                    out=eq_v, in0=eq_v,
                    in1=midxf[:].rearrange("p c -> p () c").to_broadcast([128, 5, 64]),
                    op=ALU.mult)
                g5f = msb.tile([128, 5], F32, tag="g5f")
                nc.vector.tensor_reduce(out=g5f[:], in_=eq_v,
                                        axis=mybir.AxisListType.X, op=ALU.add)
                g5u = msb.tile([128, 5], U32, tag="g5u")
                nc.vector.tensor_copy(g5u[:], g5f[:])
                # z rows (6 raw feats + lag_bias + pad) for the 5 winners
                zb = msb.tile([128, 5 * 8], F32, tag="zb")
                for r in range(K):
                    nc.gpsimd.indirect_dma_start(
                        out=zb[:, r * 8:(r + 1) * 8], out_offset=None,
                        in_=d_xzb[:, :],
                        in_offset=IndirectOffsetOnAxis(ap=g5u[:, r:r + 1], axis=0))
                vb5 = msb.tile([128, 5], F32, tag="vb5")
                nc.vector.tensor_tensor(out=vb5[:], in0=v8m[:, 0:5],
                                        in1=zb[:, 6::8], op=ALU.add)
                # softmax over the 5 candidate scores
                mx = msb.tile([128, 1], F32, tag="mx")
                nc.vector.tensor_reduce(out=mx[:], in_=vb5[:],
                                        axis=mybir.AxisListType.X, op=ALU.max)
                nmx = msb.tile([128, 1], F32, tag="nmx")
                nc.vector.tensor_scalar(out=nmx[:], in0=mx[:],
                                        scalar1=-1.0, scalar2=None, op0=ALU.mult)
                e5 = msb.tile([128, 5], F32, tag="e5")
                nc.scalar.activation(e5[:], vb5[:], AF.Exp, bias=nmx[:, 0:1])
                ssum = msb.tile([128, 1], F32, tag="ssum")
                nc.vector.tensor_reduce(out=ssum[:], in_=e5[:],
                                        axis=mybir.AxisListType.X, op=ALU.add)
                rs = msb.tile([128, 1], F32, tag="rs")
                nc.vector.reciprocal(rs[:], ssum[:])
                w5 = msb.tile([128, 5], F32, tag="w5")
                nc.vector.tensor_scalar(out=w5[:], in0=e5[:],
                                        scalar1=rs[:, 0:1], scalar2=None, op0=ALU.mult)
                # z_agg = sum_r w_r * z_r ; feat = [z_agg, z_0]
                wz = msb.tile([128, 5 * 6], F32, tag="wz")
                zview = zb[:].rearrange("p (r w) -> p r w", w=8)[:, :, 0:6]
                nc.vector.tensor_tensor(
                    out=wz[:].rearrange("p (r f) -> p r f", f=6),
                    in0=zview,
                    in1=w5[:].rearrange("p r -> p r ()").to_broadcast([128, 5, 6]),
                    op=ALU.mult)
                feat = msb.tile([128, 2 * F], F32, tag="feat")
                nc.vector.tensor_reduce(
                    out=feat[:, 0:6],
                    in_=wz[:].rearrange("p (r f) -> p f r", f=6),
                    axis=mybir.AxisListType.X, op=ALU.add)
                nc.vector.tensor_copy(feat[:, 6:12], zb[:, 0:6])
                # MLP head (tiny fp32 matmuls)
                pft = mps.tile([2 * F, 128], F32, tag="pft", space="PSUM")
                nc.tensor.transpose(out=pft[0:2 * F, 0:128], in_=feat[:, :],
                                    identity=ident[:])
                featT = msb.tile([2 * F, 128], F32, tag="featT")
                nc.scalar.activation(featT[:], pft[0:2 * F, 0:128], AF.Copy)
                ph1 = mps.tile([64, 128], F32, tag="ph1", space="PSUM")
                nc.tensor.matmul(out=ph1[:], lhsT=w1t[:], rhs=featT[:], start=True, stop=True)
                h1 = msb.tile([64, 128], F32, tag="h1")
                nc.scalar.activation(h1[:], ph1[:], AF.Relu, bias=b1c[:, 0:1])
                ph2 = mps.tile([32, 128], F32, tag="ph2", space="PSUM")
                nc.tensor.matmul(out=ph2[:], lhsT=w2t[:], rhs=h1[:], start=True, stop=True)
                h2 = msb.tile([32, 128], F32, tag="h2")
                nc.scalar.activation(h2[:], ph2[:], AF.Relu, bias=b2c[:, 0:1])
                py_ = mps.tile([1, 128], F32, tag="py", space="PSUM")
                nc.tensor.matmul(out=py_[:], lhsT=w3t[:], rhs=h2[:], start=True, stop=True)
                yrow = msb.tile([1, 128], F32, tag="yrow")
                nc.scalar.activation(yrow[:], py_[:], AF.Identity, bias=b3c[0:1, 0:1])
                pyt = mps.tile([128, 1], F32, tag="pyt", space="PSUM")
                nc.tensor.transpose(out=pyt[:, 0:1], in_=yrow[0:1, :],
                                    identity=ident[0:1, 0:1])
                ycol = msb.tile([128, 1], F32, tag="ycol")
                nc.vector.tensor_copy(ycol[:], pyt[:, 0:1])
                nc.sync.dma_start(out=d_y[j * 128:(j + 1) * 128, :], in_=ycol[:])

        big.release()
        cpool.release()

    nc.compile()
    return nc


_CACHED_NC = None


def _prep_inputs(X_scaled, X_raw, target_idx, lstm_Wih, lstm_Whh, lstm_bih,
                 lstm_bhh, ln_g, ln_b, WQ, WK, log_temp, lag_bias,
                 W1, b1, W2, b2, W3, b3):
    f32 = np.float32
    assert np.all(np.asarray(ln_b) == 0.0), "kernel assumes ln_b == 0"
    tix = np.asarray(target_idx).astype(np.int64)

    # distinct targets grouped by owner core; local stock order permuted so
    # the distinct-target stocks occupy slots 0..cnt-1
    dist = np.unique(tix)                          # sorted -> grouped by core
    perm = np.zeros((ND, SS), np.int64)            # slot -> local stock id
    dist_slot = np.full(S, -1, np.int64)           # stock -> global dist slot
    for dd in range(ND):
        loc = dist[(dist >= dd * SS) & (dist < (dd + 1) * SS)] - dd * SS
        cnt = loc.size
        assert cnt <= B, f"core {dd} has {cnt} distinct targets (> {B})"
        rest = np.setdiff1d(np.arange(SS), loc)
        perm[dd] = np.concatenate([loc, rest])
        dist_slot[dd * SS + loc] = dd * B + np.arange(cnt)
    tgt_slot = dist_slot[tix]                      # target -> global dist slot
    assert np.all(tgt_slot >= 0)

    bias = (np.asarray(lstm_bih) + np.asarray(lstm_bhh)).astype(f32)
    gperm = np.r_[0:N, N:2 * N, 3 * N:4 * N, 2 * N:3 * N]    # [i, f, o, g]
    g_ln = np.asarray(ln_g).astype(f32)
    wq_f = (np.asarray(WQ) * g_ln[None, :]).astype(f32)
    wk_f = (np.asarray(WK) * g_ln[None, :]).astype(f32)
    uq = np.asarray(WQ) @ g_ln
    uk = np.asarray(WK) @ g_ln
    inv_temp = np.asarray(
        1.0 / np.clip(np.exp(np.asarray(log_temp, np.float64)), 0.1, np.sqrt(N)),
        f32).reshape(1, 1)

    # XZB table: flat (core, local col) -> [6 raw feats at lag_pos, lag_bias, 0]
    Xr = np.asarray(X_raw)[0].astype(f32)                    # [S, L, F]
    lb = np.asarray(lag_bias).astype(f32)
    lagpos = np.clip(L - 1 - (LMAX - np.arange(LMAX)), 0, L - 1)
    stock_of_col = (np.arange(S * LMAX) // LMAX)             # flat -> perm slot
    core_of_col = stock_of_col // SS
    stock_perm = (perm + (np.arange(ND) * SS)[:, None]).reshape(-1)  # global
    xzb = np.zeros((S * LMAX, 8), f32)
    xzb[:, 0:6] = Xr[stock_perm[stock_of_col], lagpos[np.arange(S * LMAX) % LMAX]]
    xzb[:, 6] = np.tile(lb, S)

    # self-column center per (row p, tile T): tiles 2d, 2d+1 hold core d's
    # targets; on core d the target's own stock sits at score column
    # 10*slot + 4.5 (slot == dist slot by construction)
    selflo = np.full((128, NT, ND), -1e9, f32)
    for dd in range(ND):
        for j in range(2):
            sl = np.arange(128) + 128 * j
            selflo[:, 2 * dd + j, dd] = sl * LMAX + 4.5

    import ml_dtypes
    bf = ml_dtypes.bfloat16
    Xs = np.asarray(X_scaled)[0].astype(f32)                 # [S, L, F]
    # fp32 weights fold the exact bias via xt's ones-row; the bf16 weights
    # carry the bias as a bf16 (hi, lo) pair over TWO ones-rows, since a
    # single bf16-rounded bias is enough noise to flip a top-5 selection
    wih_t = np.ascontiguousarray(np.vstack([
        np.asarray(lstm_Wih).astype(f32).T, bias[None, :]])[:, gperm])
    whh_t = np.ascontiguousarray(np.asarray(lstm_Whh).astype(f32).T[:, gperm])
    bias_hi = bias[gperm].astype(bf)
    bias_lo = (bias[gperm] - bias_hi.astype(f32)).astype(bf)
    wihb_t = np.vstack([wih_t[:F].astype(bf), bias_hi[None, :], bias_lo[None, :]])
    whhb_t = whh_t.astype(bf)
    # g-gate columns x2 (exact in bf16): early steps compute tanh(g) as
    # 2*sigmoid(2g) - 1 so o and g share one packed sigmoid
    wihb_t[:, 3 * N:] = wihb_t[:, 3 * N:] * bf(2.0)
    whhb_t[:, 3 * N:] = whhb_t[:, 3 * N:] * bf(2.0)
    common = dict(
        wih_t=wih_t, whh_t=whh_t,
        wihb_t=np.ascontiguousarray(wihb_t), whhb_t=np.ascontiguousarray(whhb_t),
        wq_t=np.ascontiguousarray(wq_f.T), wk_t=np.ascontiguousarray(wk_f.T),
        negu_q=np.ascontiguousarray((-uq.astype(f32) / N).reshape(1, N)),
        negu_k=np.ascontiguousarray((-uk.astype(f32) / N).reshape(1, N)),
        invt=inv_temp, xzb=xzb,
        w1_t=np.ascontiguousarray(np.asarray(W1).astype(f32).T),
        w2_t=np.ascontiguousarray(np.asarray(W2).astype(f32).T),
        w3_t=np.ascontiguousarray(np.asarray(W3).astype(f32).T),
        b1c=np.asarray(b1).astype(f32).reshape(64, 1),
        b2c=np.asarray(b2).astype(f32).reshape(32, 1),
        b3c=np.asarray(b3).astype(f32).reshape(1, 1),
    )
    in_maps = []
    for d in range(ND):
        Xd = Xs[d * SS:(d + 1) * SS][perm[d]]                # [SS, L, F] permuted
        xtv = np.vstack([Xd.transpose(2, 1, 0).reshape(F, L * SS),
                         np.ones((1, SS * L), f32)])
        xtb_d = np.vstack([xtv[:F, :T0 * SS], np.ones((2, T0 * SS), f32)])
        in_maps.append(dict(
            common,
            xtb=np.ascontiguousarray(xtb_d).astype(bf),
            xt=np.ascontiguousarray(xtv[:, T0 * SS:]),
            selflo=np.ascontiguousarray(selflo[:, :, d]),
        ))
    return in_maps, tgt_slot


def kernel(**inputs):
    global _CACHED_NC
    if _CACHED_NC is None:
        _CACHED_NC = build_program()
    nc = _CACHED_NC
    in_maps, tgt_slot = _prep_inputs(**inputs)
    res = run_bass_kernel_spmd(nc, in_maps, core_ids=list(range(ND)))
    ydist = np.concatenate([res.results[d]["y"][:, 0] for d in range(ND)])
    return ydist[tgt_slot].astype(np.float32)
